# revision 39
# baseline (speedup 1.0000x reference)
"""MultiHeadAttention forward on 8 TRN2 NeuronCores.

Sharding: core c -> (batch b = c//2, query-half qh = c%2). Each core computes
the full attention output for 1024 query rows of one batch element (all 16
heads); outputs are disjoint slices, no collective needed.

All PE matmuls run in bf16 (precision budget 2e-2 >> bf16 error ~1e-2/sqrt).
q/k/V stay SBUF-resident (no DRAM spill). The per-head attention loop is
ACT(exp)-bound on its own (1071ns/kt vs 854ns PE), so projection and
output-projection matmuls are interleaved as "filler" PE work to keep the
tensor engine streaming continuously at peak pstate.

Per-core math (dim-on-partitions activation layout):
  qT = wqT.T @ inQ + bq          [1024, 1024] bf16   (per-hp 128-row tiles)
  kT = wkT.T @ inT + bk          [1024, 2048] bf16
  V  = inT.T @ wvT               [2048, 16*65] bf16  (65th col = ones -> denom)
  per head h: sT = kT_h.T @ qT_h            [2048, 1024] strips of [128, 1024]
              e  = exp(sT*0.125 + maskbias) (ACT, bf16 out)
              ctxT_aug = V_aug_h.T @ e      [65, 1024]; row 64 = softmax denom
              ctxT = ctxT_aug[0:64] * bcast(1/denom)
  out = ctxT_all.T @ woT + (bo + bv@wo.T)   [1024, 1024] f32

Head order 2,3,...,15,0,1 so the output projection's dt=1..6 partial sums
(contraction over dim tiles) become late-phase filler work; dt7+dt0 finish
after the last head.
"""

import numpy as np
import ml_dtypes

import concourse.bacc as bacc
import concourse.tile as tile
import concourse.mybir as mybir
from concourse.bass_utils import run_bass_kernel_spmd

F32 = mybir.dt.float32
BF16 = mybir.dt.bfloat16
EXP = mybir.ActivationFunctionType.Exp

BS, QLEN, DIM, H, DPH = 4, 2048, 1024, 16, 64
NC_ = 8
LQ = 1024  # local query rows per core
NPOS = 16  # heads, processed in order (pos+2)%16
MS = 18    # micro-slots per head: 16 score/exp slots + 2 ctx drain slots

_PROG = None
import os
DEBUG_DUMPS = bool(os.environ.get("KDBG"))


class _Sched:
    """EDF scheduler for filler matmul groups pumped into the B-phase.

    A group = consecutive matmuls accumulating in one PSUM bank + a drain op.
    At most 2 groups open at a time (2 filler PSUM banks).
    """

    def __init__(self):
        self.groups = []   # dicts: rel, dl, units (list of closures), post
        self.open = []     # groups mid-emission
        self.done_units = 0
        self.total_units = 0

    def add(self, rel, dl, units, post=None):
        self.groups.append({"rel": rel, "dl": dl, "units": list(units),
                            "post": post, "started": False})
        self.total_units += len(units)

    def seal(self):
        self.groups.sort(key=lambda g: (g["dl"], g["rel"]))

    def _avail(self, slot):
        # next unopened group that is released
        for g in self.groups:
            if not g["started"] and g["rel"] <= slot:
                return g
        return None

    def _pop_one(self, slot):
        # prefer the open group with earliest deadline; open a new one if room
        cand = None
        if len(self.open) < 2:
            cand = self._avail(slot)
        pick = None
        for g in self.open:
            if pick is None or g["dl"] < pick["dl"]:
                pick = g
        if cand is not None and (pick is None or cand["dl"] < pick["dl"]):
            pick = cand
        if pick is None:
            return False
        if not pick["started"]:
            pick["started"] = True
            self.open.append(pick)
        fn = pick["units"].pop(0)
        fn()
        self.done_units += 1
        if not pick["units"]:
            if pick["post"] is not None:
                pick["post"]()
            self.open.remove(pick)
            self.groups.remove(pick)
            if slot > pick["dl"]:
                raise ValueError(
                    f"filler group missed deadline: dl={pick['dl']} slot={slot}")
        return True

    def pump(self, slot, want=2, cap=4):
        # raise `want` if upcoming deadlines demand a higher rate
        rem = self.total_units - self.done_units
        if rem == 0:
            return
        need = want
        acc = 0
        for g in self.groups:
            acc += len(g["units"])
            slots_left = g["dl"] - slot
            if slots_left <= 0:
                need = cap
                break
            rate = (acc + slots_left - 1) // slots_left
            if rate > need:
                need = min(cap, rate)
        n = 0
        while n < need and self._pop_one(slot):
            n += 1

    def flush(self):
        while self._pop_one(10 ** 9):
            pass


def _build():
    nc = bacc.Bacc("TRN2", target_bir_lowering=False, debug=False, num_devices=NC_)

    INT = nc.dram_tensor("inT", [DIM, QLEN], BF16, kind="ExternalInput").ap()
    WQT = nc.dram_tensor("wqT", [DIM, DIM], BF16, kind="ExternalInput").ap()
    WKT = nc.dram_tensor("wkT", [DIM, DIM], BF16, kind="ExternalInput").ap()
    WVT = nc.dram_tensor("wvT", [DIM, DIM], BF16, kind="ExternalInput").ap()
    WOT = nc.dram_tensor("woT", [DIM, DIM], BF16, kind="ExternalInput").ap()
    BQC = nc.dram_tensor("bqc", [DIM, 1], F32, kind="ExternalInput").ap()
    BKC = nc.dram_tensor("bkc", [DIM, 1], F32, kind="ExternalInput").ap()
    BOR = nc.dram_tensor("boR", [1, DIM], BF16, kind="ExternalInput").ap()
    MBC = nc.dram_tensor("mb", [QLEN, 1], F32, kind="ExternalInput").ap()
    OUT = nc.dram_tensor("out", [LQ, DIM], BF16, kind="ExternalOutput").ap()
    if DEBUG_DUMPS:
        DQ0 = nc.dram_tensor("dq0", [128, LQ], BF16, kind="ExternalOutput").ap()
        DK0 = nc.dram_tensor("dk0", [128, QLEN], BF16, kind="ExternalOutput").ap()
        DV0 = nc.dram_tensor("dv0", [128, H * 65], BF16, kind="ExternalOutput").ap()
        DCTX = nc.dram_tensor("dctx", [8 * 128, LQ], BF16, kind="ExternalOutput").ap()
        DSTG = nc.dram_tensor("dstg", [65, LQ], F32, kind="ExternalOutput").ap()
        DRCP = nc.dram_tensor("drcp", [1, LQ], BF16, kind="ExternalOutput").ap()

    with tile.TileContext(nc) as tc:
        from contextlib import ExitStack
        with ExitStack() as ctx:
            const_p = ctx.enter_context(tc.tile_pool(name="const", bufs=1))
            qkv_p = ctx.enter_context(tc.tile_pool(name="qkv", bufs=1))
            ctx_p = ctx.enter_context(tc.tile_pool(name="ctxall", bufs=1))
            w_p = ctx.enter_context(tc.tile_pool(name="wstream", bufs=1))
            stg_p = ctx.enter_context(tc.tile_pool(name="stg", bufs=2))
            e_p = ctx.enter_context(tc.tile_pool(name="ex", bufs=3))
            pss = ctx.enter_context(tc.tile_pool(name="pss", bufs=2, space="PSUM"))
            psc = ctx.enter_context(tc.tile_pool(name="psc", bufs=1, space="PSUM"))
            psf = ctx.enter_context(tc.tile_pool(name="psf", bufs=2, space="PSUM"))
            dram_p = ctx.enter_context(tc.tile_pool(name="dram", bufs=1, space="DRAM"))
            # inT pool closed mid-build (after V-oc1 fillers) to free SBUF
            inp_cm = tc.tile_pool(name="inp", bufs=1)
            inp = inp_cm.__enter__()

            # ---- constants + small DMAs (scalar queue is idle pre-B) ----
            ones_f = const_p.tile([1, 128], F32, tag="onesf")
            nc.vector.memset(ones_f[:], 1.0)
            ones_b = const_p.tile([1, 128], BF16, tag="onesb")
            nc.vector.tensor_copy(ones_b[:], ones_f[:])
            bq_t = const_p.tile([128, 8], F32, tag="bq")
            nc.sync.dma_start(bq_t[:], BQC.rearrange("(g p) o -> p (g o)", p=128))
            bk_t = const_p.tile([128, 8], F32, tag="bk")
            nc.sync.dma_start(bk_t[:], BKC.rearrange("(g p) o -> p (g o)", p=128))
            mb_t = const_p.tile([128, 16], F32, tag="mb")
            nc.sync.dma_start(mb_t[:], MBC.rearrange("(g p) o -> p (g o)", p=128))
            bo_r = const_p.tile([1, DIM], BF16, tag="bor")
            nc.scalar.dma_start(bo_r[:], BOR[:])

            # ---- persistent SBUF tensors ----
            # inT columns are host-rotated so cols 0:1024 are the LOCAL query
            # half: q-projection reads them directly (key order is a per-core
            # permutation; attention sums over keys are permutation-invariant)
            inT = inp.tile([128, 8, QLEN], BF16, tag="inT", name="inT")
            # qTz[h]: per-head zero-padded q so score matmuls use a full
            # 128-row moving operand (uniform (128,128) PE tile config; the
            # 64<->128 row-mode switch costs ~100ns per transition)
            qTz = [qkv_p.tile([128, LQ], BF16, tag=f"q{h}", name=f"qTz{h}") for h in range(16)]
            kT = [qkv_p.tile([128, QLEN], BF16, tag=f"k{j}", name=f"kT{j}") for j in range(8)]
            V_sb = [qkv_p.tile([128, H * 65], BF16, tag=f"v{st}", name=f"v{st}") for st in range(16)]
            ctx_all = [ctx_p.tile([128, LQ], BF16, tag=f"c{dt}", name=f"ctx{dt}") for dt in range(8)]
            for st in range(16):
                nc.vector.memset(V_sb[st][:, 64::65], 1.0)


            # ---- input DMAs: column-strip order so the first kproj group
            # (which accumulates over ALL 8 it-tiles of one 512-col strip) can
            # start after ~1.25MB instead of after the full 4MB inT load.
            # Keep the vector queue free so prelude PSUM drains are not blocked.
            intr = INT.rearrange("(it p) m -> p it m", p=128)
            # consumption order: kproj(1,sc0) -> qproj(1,oc0) -> kproj(1,sc1)
            # -> qproj(1,oc1) -> vproj(0,st).  Each strip is split across the
            # sync (it 0-3) and gpsimd (it 4-7) queues so the first group
            # unblocks at 2x queue bandwidth.
            # sc0/sc1 it0-3 on sync; it4-7 on gpsimd; sc2/sc3 (only needed by
            # the prelude V st8-15 groups) split gpsimd / scalar-after-wv0
            for sc in range(4):
                for it in range(4):
                    if sc < 2:
                        nc.sync.dma_start(inT[:, it, sc * 512:(sc + 1) * 512],
                                          intr[:, it, sc * 512:(sc + 1) * 512])
                for it in range(4, 8):
                    nc.gpsimd.dma_start(inT[:, it, sc * 512:(sc + 1) * 512],
                                        intr[:, it, sc * 512:(sc + 1) * 512])

            # ---- weight DMAs ----
            # scalar queue (idle until B): hp=1 weights + both wv halves
            def load_w(engine, W, j, tag):
                t = w_p.tile([128, 8, 128], BF16, tag=tag, bufs=3)
                engine.dma_start(
                    t[:], W[:, j * 128:(j + 1) * 128].rearrange("(it p) m -> p it m", p=128))
                return t

            wq_t = {}
            wk_t = {}
            wk_t[1] = load_w(nc.scalar, WKT, 1, "wk")
            wq_t[1] = load_w(nc.scalar, WQT, 1, "wq")
            # wv0 chunked so the prelude V groups never wait on a monolithic
            # 8KB/partition load
            wv_t = [w_p.tile([128, 8, 512], BF16, tag="wv", bufs=2, name=f"wv{oc}")
                    for oc in range(2)]
            wvr = [WVT[:, oc * 512:(oc + 1) * 512].rearrange("(it p) m -> p it m", p=128)
                   for oc in range(2)]
            nc.scalar.dma_start(wv_t[0][:, 0:2, :], wvr[0][:, 0:2, :])
            nc.scalar.dma_start(wv_t[0][:, 2:5, :], wvr[0][:, 2:5, :])
            nc.scalar.dma_start(wv_t[0][:, 5:8, :], wvr[0][:, 5:8, :])
            for sc in range(2, 4):
                for it in range(4):
                    nc.scalar.dma_start(inT[:, it, sc * 512:(sc + 1) * 512],
                                        intr[:, it, sc * 512:(sc + 1) * 512])
            nc.scalar.dma_start(wv_t[1][:], wvr[1][:])
            # sync queue: weights in true CONSUMPTION order.  kproj has early
            # (exchange-issue) deadlines, qproj keeps pos-based ones -- the
            # queue and each tag's buffer-release chain must both be monotone
            # in consumption order or the stream deadlocks.
            wk_t[2] = load_w(nc.sync, WKT, 2, "wk")
            wq_t[2] = load_w(nc.sync, WQT, 2, "wq")
            for j in [3, 4, 5, 6, 7, 0]:
                wk_t[j] = load_w(nc.sync, WKT, j, "wk")
            for j in [3, 4, 0, 5, 6, 7]:
                wq_t[j] = load_w(nc.sync, WQT, j, "wq")
            # wo tiles are allocated + DMA'd at pos 8, into the space freed
            # by the inT pool (they are only needed from ~slot 222)
            wo_t = []

            # ---- filler group emitters ----
            def qproj_group(j, oc):
                f = psf.tile([128, 512], F32, tag="f", name="fq")
                units = []
                for it in range(8):
                    units.append(lambda it=it, f=f: nc.tensor.matmul(
                        f[:], wq_t[j][:, it, :], inT[:, it, oc * 512:(oc + 1) * 512],
                        start=(it == 0), stop=(it == 7)))
                def post(f=f):
                    # scatter the two heads' halves into their zero-padded
                    # tiles; the first post also zeroes the dead halves
                    if oc == 0:
                        nc.vector.memset(qTz[2 * j][64:128, :], 0.0)
                        nc.vector.memset(qTz[2 * j + 1][0:64, :], 0.0)
                    nc.vector.tensor_scalar_add(
                        qTz[2 * j][0:64, oc * 512:(oc + 1) * 512],
                        f[0:64, :], bq_t[0:64, j:j + 1])
                    nc.vector.tensor_scalar_add(
                        qTz[2 * j + 1][64:128, oc * 512:(oc + 1) * 512],
                        f[64:128, :], bq_t[64:128, j:j + 1])
                return units, post

            def kproj_group(j, sc):
                f = psf.tile([128, 512], F32, tag="f", name="fk")
                units = []
                for it in range(8):
                    units.append(lambda it=it, f=f: nc.tensor.matmul(
                        f[:], wk_t[j][:, it, :], inT[:, it, sc * 512:(sc + 1) * 512],
                        start=(it == 0), stop=(it == 7)))
                def post(f=f):
                    nc.vector.tensor_scalar_add(
                        kT[j][:, sc * 512:(sc + 1) * 512], f[:], bk_t[:, j:j + 1])
                return units, post

            def vproj_group(oc, st):
                f = psf.tile([128, 512], F32, tag="f", name="fv")
                units = []
                for it in range(8):
                    units.append(lambda it=it, f=f: nc.tensor.matmul(
                        f[:], inT[:, it, st * 128:(st + 1) * 128], wv_t[oc][:, it, :],
                        start=(it == 0), stop=(it == 7)))
                def post(f=f, oc=oc, st=st):
                    dst = V_sb[st][:].rearrange("p (h c) -> p h c", c=65)
                    nc.vector.tensor_copy(
                        dst[:, oc * 8:(oc + 1) * 8, 0:64],
                        f[:].rearrange("p (h c) -> p h c", c=64))
                return units, post

            # bobc: broadcast output bias row to 128 partitions (f32 SBUF)
            bobc = const_p.tile([128, DIM], BF16, tag="bobc")

            def bobc_half(oc):
                f = psf.tile([128, 512], F32, tag="f", name="fbo")
                units = [lambda oc=oc, f=f: nc.tensor.matmul(
                    f[:], ones_b[:], bo_r[:, oc * 512:(oc + 1) * 512],
                    start=True, stop=True)]
                def post(f=f, oc=oc):
                    nc.vector.tensor_copy(bobc[:, oc * 512:(oc + 1) * 512], f[:])
                return units, post

            # ---- prelude: emit directly (PE busy while inputs stream in) ----
            def emit_group(units, post):
                for u in units:
                    u()
                post()

            emit_group(*bobc_half(0))
            emit_group(*bobc_half(1))
            emit_group(*kproj_group(1, 0))
            emit_group(*qproj_group(1, 0))
            emit_group(*kproj_group(1, 1))
            emit_group(*qproj_group(1, 1))
            for st in range(16):
                emit_group(*vproj_group(0, st))

            # ---- build filler schedule for phase B ----
            sched = _Sched()
            # Local-half K projections feed the pair-wise AllGather exchanges:
            # deadlines are set by each chunk's collective issue slot, not by
            # first score use (the remote half arrives over the wire).
            KDL = {2: 16, 3: 24, 4: 24, 5: 70, 6: 70, 7: 70, 0: 70}
            for j in [2, 3, 4, 5, 6, 7, 0]:
                for sc in range(2):
                    sched.add(0, KDL[j], *kproj_group(j, sc))
            # V-oc1 local half (st 0-7) feeds the V exchange at pos3
            for st in range(8):
                sched.add(4, 30, *vproj_group(1, st))
            pos0_dl = 8 * MS - 1
            for j in [2, 3, 4, 5, 6, 7]:
                dl = (2 * j - 2) * MS
                for oc in range(2):
                    sched.add(0, dl, *qproj_group(j, oc))
            for oc in range(2):
                sched.add(0, pos0_dl, *qproj_group(0, oc))

            # output-projection partials over dt=1..6 (ready after pos 11's
            # normalize ~ slot 12*MS+6); drain folds in the output bias
            cpart_cm = None
            cpart = []

            def cgroup(st, oc):
                f = psf.tile([128, 512], F32, tag="f", name="fc")
                units = []
                for dt in range(1, 7):
                    units.append(lambda dt=dt, f=f, st=st, oc=oc: nc.tensor.matmul(
                        f[:], ctx_all[dt][:, st * 128:(st + 1) * 128],
                        wo_t[dt][:, oc * 512:(oc + 1) * 512],
                        start=(dt == 1), stop=(dt == 6)))
                def post(f=f, st=st, oc=oc):
                    nc.vector.tensor_add(
                        cpart[st * 2 + oc][:], f[:], bobc[:, oc * 512:(oc + 1) * 512])
                return units, post

            # ---- pair-wise K/V-oc1 exchange machinery ----
            # Each core computes only the LOCAL key-half of K (cols 0:1024)
            # and of V-oc1 (heads 8-15, st 0-7); the other half arrives via
            # pair AllGathers.  cond-DMAs (on cc_rank parity) pick the
            # partner block -- the skipped DMA still fires its semaphore so
            # dependency tracking stays sound.
            CCG = [[0, 1], [2, 3], [4, 5], [6, 7]]
            K_CHUNKS = [[1], [2], [3, 4], [5, 6, 7, 0]]
            kb_in, kb_out = [], []
            for ci, js in enumerate(K_CHUNKS):
                kb_in.append(dram_p.tile([len(js), 128, 1024], BF16, tag=f"kin{ci}", name=f"kin{ci}"))
                kb_out.append(dram_p.tile([2, len(js), 128, 1024], BF16, tag=f"kout{ci}", name=f"kout{ci}"))
            vb_in = dram_p.tile([8, 128, 520], BF16, tag="vin", name="vbin")
            vb_out = dram_p.tile([2, 8, 128, 520], BF16, tag="vout", name="vbout")

            def k_issue(ci):
                js = K_CHUNKS[ci]
                for x, j in enumerate(js):
                    nc.gpsimd.dma_start(kb_in[ci][x], kT[j][:, 0:1024])
                nc.gpsimd.collective_compute(
                    "AllGather", mybir.AluOpType.bypass, replica_groups=CCG,
                    ins=[kb_in[ci][:].opt()], outs=[kb_out[ci][:].opt()])

            def k_back(ci):
                rk = nc.scalar.cc_rank(CCG)
                for x, j in enumerate(K_CHUNKS[ci]):
                    nc.scalar.dma_start(kT[j][:, 1024:2048], kb_out[ci][1, x],
                                        cond=(rk == 0))
                    nc.scalar.dma_start(kT[j][:, 1024:2048], kb_out[ci][0, x],
                                        cond=(rk != 0))

            def v_issue():
                for st in range(8):
                    nc.gpsimd.dma_start(vb_in[st], V_sb[st][:, 520:1040])
                nc.gpsimd.collective_compute(
                    "AllGather", mybir.AluOpType.bypass, replica_groups=CCG,
                    ins=[vb_in[:].opt()], outs=[vb_out[:].opt()])

            def v_back():
                rk = nc.scalar.cc_rank(CCG)
                for st in range(8):
                    nc.scalar.dma_start(V_sb[8 + st][:, 520:1040], vb_out[1, st],
                                        cond=(rk == 0))
                    nc.scalar.dma_start(V_sb[8 + st][:, 520:1040], vb_out[0, st],
                                        cond=(rk != 0))

            sched.seal()
            k_issue(0)  # kT[1] local half is ready at prelude end

            # ---- phase B: attention heads with interleaved fillers ----
            prev = None  # (hp, half, stg tile) awaiting denominator/normalize
            for pos in range(NPOS):
                h = (pos + 2) % 16
                hp, half = h // 2, h % 2
                if pos == 8:
                    # inT fully consumed (V-oc1 deadline was pos 6):
                    # free its SBUF, then allocate wo tiles + C partial tiles
                    inp_cm.__exit__(None, None, None)
                    cpart_cm = tc.tile_pool(name="cpart", bufs=1)
                    cp_pool = cpart_cm.__enter__()
                    for dt in range(8):
                        wo_t.append(cp_pool.tile([128, DIM], BF16, tag=f"wo{dt}",
                                                 name=f"wo{dt}"))
                        nc.sync.dma_start(wo_t[dt][:],
                                           WOT[dt * 128:(dt + 1) * 128, :])
                    for i in range(16):
                        # bf16: rounds the dt1..6 partial (~0.3% of output),
                        # well inside the 2e-2 budget; halves SBUF footprint
                        cpart.append(cp_pool.tile([128, 512], BF16, tag=f"cp{i}",
                                                  name=f"cp{i}"))
                    # staggered releases spread C work over the B tail so the
                    # last slots are not left without filler matmuls
                    for st in range(8):
                        for oc in range(2):
                            sched.add(12 * MS + 8 + 4 * (st * 2 + oc),
                                      NPOS * MS - 1, *cgroup(st, oc))

                ps_ctx = psc.tile([65, LQ], F32, tag="c", name="psctx")
                exs = [None] * 16
                for ms in range(MS):
                    slot = pos * MS + ms
                    if (pos, ms) == (0, 2):
                        k_back(0)
                    elif (pos, ms) == (1, 2):
                        k_issue(1)
                    elif (pos, ms) == (1, 12):
                        k_issue(2)
                    elif (pos, ms) == (2, 2):
                        v_issue()
                    elif (pos, ms) == (2, 6):
                        k_back(1)
                    elif (pos, ms) == (3, 8):
                        k_back(2)
                    elif (pos, ms) == (4, 2):
                        k_issue(3)
                    elif (pos, ms) == (4, 6):
                        v_back()
                    elif (pos, ms) == (5, 0):
                        k_back(3)
                    if ms < 16:
                        kt = ms
                        ps_s = pss.tile([128, LQ], F32, tag="s", name="pss")
                        for qc in range(2):
                            nc.tensor.matmul(
                                ps_s[:, qc * 512:(qc + 1) * 512],
                                kT[hp][:, kt * 128:(kt + 1) * 128],
                                qTz[h][:, qc * 512:(qc + 1) * 512],
                                start=True, stop=True)
                        ex = e_p.tile([128, LQ], BF16, tag="ex", name="ex")
                        nc.scalar.activation(ex[:], ps_s[:], EXP,
                                             bias=mb_t[:, kt:kt + 1], scale=0.125)
                        exs[kt] = ex
                    # previous head's normalize: reciprocal of the denominator
                    # row, broadcast across partitions on GpSimd (not the PE)
                    if prev is not None and ms == 3:
                        php, phalf, pstg, pdrow = prev
                        rcpf = stg_p.tile([1, LQ], F32, tag="rcpf", name="rcpf", bufs=1)
                        nc.vector.reciprocal_approx_fast(rcpf[:], pdrow[:])
                        bc = stg_p.tile([64, LQ], F32, tag="bc", name="bc", bufs=1)
                        nc.gpsimd.partition_broadcast(bc[:], rcpf[:])
                        nc.vector.tensor_mul(
                            ctx_all[php][phalf * 64:(phalf + 1) * 64, :],
                            pstg[0:64, :], bc[:])
                        prev = None
                    sched.pump(slot, want=2, cap=4)
                    if ms >= 2:
                        kt = ms - 2
                        for qc in range(2):
                            nc.tensor.matmul(
                                ps_ctx[:, qc * 512:(qc + 1) * 512],
                                V_sb[kt][:, h * 65:(h + 1) * 65],
                                exs[kt][:, qc * 512:(qc + 1) * 512],
                                start=(kt == 0), stop=(kt == 15))
                # stage ctx+denominator to SBUF (frees ps_ctx for the next
                # head), reciprocal later; the LAST head skips staging -- its
                # normalize reads ps_ctx directly after the loop
                last = pos == NPOS - 1
                stg = stg_p.tile([65, LQ], F32, tag="stg", name="stg", bufs=2)
                (nc.scalar.copy if last else nc.vector.tensor_copy)(stg[:], ps_ctx[:])
                drow = stg_p.tile([1, LQ], F32, tag="drow", name="drow", bufs=1)
                (nc.scalar.copy if last else nc.vector.tensor_copy)(drow[:], stg[64:65, :])
                prev = (hp, half, stg, drow)

            # ---- post: last head's normalize (direct from PSUM, pipelined
            # by q-halves), then the dt7/dt0 out pipeline on 6 PSUM banks ----
            sched.flush()
            # Open the tail pool and queue the first dt7 matmuls BEFORE the
            # last-head normalize chain: the pool-acquire barrier otherwise
            # serializes the whole tail behind the chain, and the PE then
            # restarts at low p-state.
            with tc.tile_pool(name="otp", bufs=3) as otp:
                fq = [None] * 16
                # 6 rotating PSUM half-banks: the two (drained) score buffers
                # provide 4, the filler pool the other 2
                fo_aps = []
                for _ in range(2):
                    t = pss.tile([128, LQ], F32, tag="s", name="fo_s")
                    fo_aps.append(t[:, 0:512])
                    fo_aps.append(t[:, 512:1024])
                for _ in range(2):
                    fo_aps.append(psf.tile([128, 512], F32, tag="f", name="fo_f")[:])

                def dt7(i):
                    st, oc = i // 2, i % 2
                    f = fo_aps[i % 6]
                    fq[i] = f
                    nc.tensor.matmul(f, ctx_all[7][:, st * 128:(st + 1) * 128],
                                     wo_t[7][:, oc * 512:(oc + 1) * 512],
                                     start=True, stop=False)

                for i in range(6):
                    dt7(i)
                # last head (head 1 -> ctx_all[0][64:128]), staged normalize
                php, phalf, pstg, pdrow = prev
                rcpf = stg_p.tile([1, LQ], F32, tag="rcpf", name="rcpf", bufs=1)
                nc.vector.reciprocal_approx_fast(rcpf[:], pdrow[:])
                bc = stg_p.tile([64, LQ], F32, tag="bc", name="bc", bufs=1)
                nc.gpsimd.partition_broadcast(bc[:], rcpf[:])
                nc.vector.tensor_mul(
                    ctx_all[php][phalf * 64:(phalf + 1) * 64, :],
                    pstg[0:64, :], bc[:])

                for j in range(16):
                    stj, ocj = j // 2, j % 2
                    nc.tensor.matmul(fq[j], ctx_all[0][:, stj * 128:(stj + 1) * 128],
                                     wo_t[0][:, ocj * 512:(ocj + 1) * 512],
                                     start=False, stop=True)
                    ot = otp.tile([128, 512], BF16, tag="ot", name="ot")
                    nc.vector.tensor_add(ot[:], fq[j], cpart[j][:])
                    # dt7(j+3) reuses bank (j-3)%6: 3 iterations of drain slack
                    if 3 <= j <= 12:
                        dt7(j + 3)
                    (nc.sync if j % 2 == 0 else nc.gpsimd).dma_start(
                        OUT[stj * 128:(stj + 1) * 128, ocj * 512:(ocj + 1) * 512],
                        ot[:])
            if DEBUG_DUMPS:
                nc.sync.dma_start(DQ0[:], qTz[0][:])
                nc.sync.dma_start(DK0[:], kT[0][:])
                nc.sync.dma_start(DV0[:], V_sb[0][:])
                for dt in range(8):
                    nc.sync.dma_start(DCTX[dt * 128:(dt + 1) * 128, :], ctx_all[dt][:])
                nc.sync.dma_start(DSTG[:], pstg[:])
                nc.sync.dma_start(DRCP[:], pdrow[:])
            if cpart_cm is not None:
                cpart_cm.__exit__(None, None, None)

    nc.compile()
    return nc


def _get_prog():
    global _PROG
    if _PROG is None:
        _PROG = _build()
    return _PROG


def _bf16(x):
    return np.ascontiguousarray(x).astype(ml_dtypes.bfloat16)


def kernel(input, mask, wq, bq, wk, bk, wv, bv, wo, bo, _trace=False):
    nc = _get_prog()

    input = np.asarray(input, np.float32)
    mask = np.asarray(mask)
    wq, bq = np.asarray(wq, np.float32), np.asarray(bq, np.float32)
    wk, bk = np.asarray(wk, np.float32), np.asarray(bk, np.float32)
    wv, bv = np.asarray(wv, np.float32), np.asarray(bv, np.float32)
    wo, bo = np.asarray(wo, np.float32), np.asarray(bo, np.float32)

    inT = [_bf16(input[b].T) for b in range(BS)]
    wqT = _bf16(wq.T)
    wkT = _bf16(wk.T)
    wvT = _bf16(wv.T)
    woT = _bf16(wo.T)
    bqc = bq.reshape(DIM, 1)
    bkc = bk.reshape(DIM, 1)
    boR = _bf16((bo + bv @ wo.T).reshape(1, DIM))
    mb = [np.where(mask[b] == 0, np.float32(-30.0), np.float32(0.0))
          .astype(np.float32).reshape(QLEN, 1) for b in range(BS)]

    in_maps = []
    for c in range(NC_):
        b, qh = c // 2, c % 2
        # rotate key/query columns so the LOCAL query half sits at cols
        # 0:1024 (fixed SPMD offset); attention is key-permutation-invariant
        # as long as the mask bias rows are permuted identically
        perm = np.r_[qh * LQ:(qh + 1) * LQ, (1 - qh) * LQ:(2 - qh) * LQ]
        in_maps.append({
            "inT": np.ascontiguousarray(inT[b][:, perm]),
            "wqT": wqT, "wkT": wkT, "wvT": wvT, "woT": woT,
            "bqc": bqc, "bkc": bkc, "boR": boR,
            "mb": np.ascontiguousarray(mb[b][perm]),
        })

    res = run_bass_kernel_spmd(nc, in_maps, list(range(NC_)), trace=_trace)

    out = np.empty((BS, QLEN, DIM), np.float32)
    for c in range(NC_):
        b, qh = c // 2, c % 2
        out[b, qh * LQ:(qh + 1) * LQ, :] = res.results[c]["out"].astype(np.float32)
    if _trace:
        kernel.last_exec_time_ns = res.exec_time_ns
    if _trace or DEBUG_DUMPS:
        kernel.last_results = res
    return out

